# revision 7
# baseline (speedup 1.0000x reference)
"""Behler-Parrinello fingerprints on 8 TRN2 cores — v2.

Key restructurings vs v1:
  - Diagonal-packed pair domain: unordered pairs (j<k) live on 12 cyclic
    diagonals -> [P, 12, 24] tensors (288 elems vs 576). k-side operands
    are overlapping strided APs into doubled [P,48] per-neighbor vectors;
    j-side operands are step-0 broadcasts. out11 needs no halving trick;
    the d=12 diagonal (double-counted) gets a 0.5 weight.
  - fc cutoff via polynomial: 1 + cos(pi*x) = 2*psi(x^2)^2 with psi a
    degree-3 polynomial in t = (d/Rc)^2 -> no Sin, no Sqrt. Every ACT op
    is Exp/Square/Copy, all in ONE table set => zero ACT table reloads.
  - distances pre-scaled by 1/Rc host-side so t = dsq directly.
  - bf16 W-weight chain (2x DVE mode); cos/pow chain stays f32 (z=16
    powers amplify input rounding x16).
  - z=1 features use affine_mul_reduce (fused (c*sc +- sc) * GW + row
    reduce); z in {2,4,16} powers via ACT Square chain directly from CC
    (Square(scale*x+bias) gives (1 +- c)^2 with no intermediate).
"""
import numpy as np
import ml_dtypes

A_TOT = 8192
N_NEI = 24
F = 8
N_CORES = 8
A_CORE = A_TOT // N_CORES      # 1024
P = 128
NTILES = A_CORE // P           # 8
ND = N_NEI // 2                # 12 diagonals

# degree-3 fit of psi(t) ~= cos(pi*sqrt(t)/2) on [0,1] (maxerr 6.8e-6)
PSI = (0.9999932539239178, -1.2334836942823042,
       0.2525780342887728, -0.019094247033354855)

_BUILT = {}


def _np_reference(n_diff, n_dist, atom_i_idx, j_elems, eta2, R_s, R_c2,
                  zeta, Lambda, eta4, R_c4, n_atoms, n_nei):
    """Pure-numpy fallback (exact reference semantics), chunked over atoms."""
    dt = np.float32
    m1 = (j_elems == 1).astype(dt)
    m8 = (j_elems == 8).astype(dt)

    def fc(d, R_c):
        return 0.5 * (np.cos(np.pi * d / R_c) + 1.0)

    d = n_dist[:, None]
    out_g2 = []
    for m in (m1, m8):
        sf = np.exp(-eta2 * (d - R_s) ** 2) * fc(d, R_c2) * m[:, None]
        acc = np.zeros((n_atoms, F), dt)
        np.add.at(acc, atom_i_idx, sf)
        out_g2.append(acc)

    diff = n_diff.reshape(n_atoms, n_nei, 3)
    dist = n_dist.reshape(n_atoms, n_nei)
    jm1 = m1.reshape(n_atoms, n_nei)
    jm8 = m8.reshape(n_atoms, n_nei)

    def g4(jm, km, same):
        res = np.zeros((n_atoms, F), dt)
        CH = 256
        for s in range(0, n_atoms, CH):
            e = min(s + CH, n_atoms)
            dj = diff[s:e] * jm[s:e][..., None]
            dk = diff[s:e] * km[s:e][..., None]
            rj = dist[s:e] * jm[s:e]
            rk = dist[s:e] * km[s:e]
            dot = np.einsum('anc,amc->anm', dj, dk)
            rp = rj[:, :, None] * rk[:, None, :]
            valid = rp > 0
            if same:
                valid = valid & np.triu(np.ones((n_nei, n_nei), bool), k=1)
            cos = dot / np.where(valid, rp, 1.0)
            sq = ((dk[:, None, :, :] - dj[:, :, None, :]) ** 2).sum(-1)
            djk = np.sqrt(np.where(sq > 0, sq, 1.0))
            djk = np.where(sq > 0, djk, 0.0)
            valid = valid & (djk < R_c4[0])
            p1 = (cos[..., None] * Lambda + 1.0) ** zeta
            p2 = np.exp(-eta4 * (rj[:, :, None] ** 2
                                 + rk[:, None, :] ** 2)[..., None])
            p3 = (fc(rj[:, :, None, None], R_c4) * fc(rk[:, None, :, None],
                                                      R_c4)
                  * fc(djk[..., None], R_c4))
            term = p1 * p2 * p3 * (2.0 ** (1.0 - zeta)) * valid[..., None]
            res[s:e] = term.sum(axis=(1, 2))
        return res

    return np.concatenate([out_g2[0], out_g2[1],
                           g4(jm1, jm8, False), g4(jm1, jm1, True)], axis=1)


# engine per op: "v"=DVE, "g"=GPSIMD, "a"=ACT(copy, where applicable)
PLAN = {
    "cxx": "v", "cyy": "v", "czz": "v", "cca": "v", "cc": "v",
    "rp": "v", "s": "v", "t1": "v", "sq": "v", "tq": "v",
    "aq": "a", "bq": "a", "u1q": "v", "psi": "v",
    "w18a": "v", "w18b": "v", "w18": "v", "w11": "v",
    "gw18": "v", "gw11": "v",
    # squares engine: a=ACT always; allow v to offload some to DVE as TT
    "sq_plan": "aaaaaaaa",
    "w_bf16": True,
    "use_affine_z1": True,
    "big_bufs": 5,
    "io_bufs": 4,
    "small_bufs": 3,
    "g2x": "v", "g2p": "v",
    "fuse_pows": True,   # pow chain as [P,2,288] paired ACT squares
    "g2_cat": True,      # one TT+reduce for both G2 weights
    "skip_nclamp": True, # per-neighbor d < Rc guaranteed by kernel() gate
    "scratch_bufs": 2,   # slots for the accum dummy-out tile
    "loop_hints": True, "loop_staggered": True,  # timing-loop back-edge cost
    "acc_ttr": False,    # accums via tensor_tensor_reduce instead of stt
}


def _build_nc(eta2, R_s, R_c2, zeta, Lambda, eta4u, R_c4u,
              loop_reps=None, plan=None, ntiles=NTILES):
    import contextlib
    import concourse.bass as bass
    import concourse.tile as tile
    from concourse import bacc, mybir

    if plan is None:
        plan = PLAN
    f32 = mybir.dt.float32
    bf16 = mybir.dt.bfloat16
    wdt = bf16 if plan["w_bf16"] else f32
    Alu = mybir.AluOpType
    Act = mybir.ActivationFunctionType
    N = N_NEI
    zi = [int(z) for z in zeta]
    assert all(abs(z - i) < 1e-6 and i >= 1 for z, i in zip(zeta, zi))
    sc = [0.125 * (2.0 ** (1.0 - z)) for z in zeta]
    rc2 = R_c4u * R_c4u
    c0, c1, c2, c3 = PSI

    nc = bacc.Bacc("TRN2", target_bir_lowering=False, debug=False)
    # dd: d/Rc doubled [A,48]; ud: unit vectors doubled [A,3*48];
    # m1/m8: masks doubled [A,48] (bf16 when w_bf16)
    dd_in = nc.dram_tensor("dd", [A_CORE, 2 * N], f32, kind="ExternalInput")
    ud_in = nc.dram_tensor("ud", [A_CORE, 6 * N], f32, kind="ExternalInput")
    m1_in = nc.dram_tensor("m1", [A_CORE, 2 * N], wdt, kind="ExternalInput")
    m8_in = nc.dram_tensor("m8", [A_CORE, 2 * N], wdt, kind="ExternalInput")
    out_dr = nc.dram_tensor("out", [A_CORE, 4 * F], f32,
                            kind="ExternalOutput")

    def eng(key):
        return {"v": nc.vector, "g": nc.gpsimd, "a": nc.scalar}[plan[key]]

    with tile.TileContext(nc) as tc:
        with (
            tc.tile_pool(name="consts", bufs=1) as consts,
            tc.tile_pool(name="io", bufs=plan.get("io_bufs", 3)) as io,
            tc.tile_pool(name="small",
                         bufs=plan.get("small_bufs", 2)) as small,
            tc.tile_pool(name="big", bufs=plan["big_bufs"]) as big,
        ):
            ln05 = consts.tile([P, 1], f32)
            nc.vector.memset(ln05[:], float(np.log(0.5)))
            # ETA[f, n] = -eta2_f * Rc^2 (const, built once)
            eta_t = consts.tile([P, F, N], f32)
            for f in range(F):
                nc.vector.memset(eta_t[:, f, :], float(-eta2[f] * rc2))

            def jb(t2, w=None):
                # [P,48] tile -> [P,12,24] j-broadcast (first 24 cols)
                return t2[:, 0:24].unsqueeze(1).broadcast_to([P, ND, 24])

            def kb(t2):
                # [P,48] tile -> [P,12,24] overlapping shifted view:
                # elem (d,j) = t2[p, 1+d+j -1] i.e. offset 1, steps (1,1)
                ap = t2[:]
                return bass.AP(ap.tensor, ap.offset + 1,
                               [list(ap.ap[0]), [1, ND], [1, 24]])

            def emit_tile(it):
                r0, r1 = it * P, (it + 1) * P
                dd = io.tile([P, 2 * N], f32, tag="dd")
                ud = io.tile([P, 3, 2 * N], f32, tag="ud")
                m1t = io.tile([P, 2 * N], wdt, tag="m1t")
                m8t = io.tile([P, 2 * N], wdt, tag="m8t")
                nc.sync.dma_start(dd[:], dd_in[r0:r1, :])
                nc.sync.dma_start(ud[:], ud_in[r0:r1, :].rearrange(
                    "p (c n) -> p c n", c=3))
                nc.sync.dma_start(m1t[:], m1_in[r0:r1, :])
                nc.sync.dma_start(m8t[:], m8_in[r0:r1, :])
                out_t = io.tile([P, 4 * F], f32, tag="out_t")

                # ---- per-neighbor tables ([P,48]; t-units: dd = d/Rc) --
                dsq = small.tile([P, 2 * N], f32, tag="dsq")
                nc.gpsimd.tensor_mul(dsq[:], dd[:], dd[:])
                if plan.get("skip_nclamp", False):
                    tqn = dsq
                else:
                    tqn = small.tile([P, 2 * N], f32, tag="tqn")
                    nc.vector.tensor_scalar(tqn[:], dsq[:], 1.0, 1.0,
                                            Alu.mult, Alu.min)
                s2n = small.tile([P, 2 * N], f32, tag="s2n")
                nc.scalar.activation(s2n[:], tqn[:], Act.Square)
                an = small.tile([P, 2 * N], f32, tag="an")
                nc.vector.tensor_scalar(an[:], tqn[:], c1, c0,
                                        Alu.mult, Alu.add)
                bn_ = small.tile([P, 2 * N], f32, tag="bn_")
                nc.vector.tensor_scalar(bn_[:], tqn[:], c3, c2,
                                        Alu.mult, Alu.add)
                u1n = small.tile([P, 2 * N], f32, tag="u1n")
                nc.vector.tensor_mul(u1n[:], s2n[:], bn_[:])
                psin = small.tile([P, 2 * N], f32, tag="psin")
                nc.vector.tensor_add(psin[:], an[:], u1n[:])
                # Bn = 2*psi^2 = 1 + cos(pi d/Rc)
                bn = small.tile([P, 2 * N], f32, tag="bn")
                nc.scalar.activation(bn[:], psin[:], Act.Square,
                                     scale=float(np.sqrt(2.0)))
                e4t = small.tile([P, 2 * N], f32, tag="e4t")
                nc.scalar.activation(e4t[:], dsq[:], Act.Exp,
                                     scale=float(-eta4u * rc2))
                base = small.tile([P, 2 * N], f32, tag="base")
                nc.vector.tensor_mul(base[:], bn[:], e4t[:])
                h1 = small.tile([P, 2 * N], wdt, tag="h1")
                nc.vector.tensor_mul(h1[:], base[:], m1t[:])
                h8 = small.tile([P, 2 * N], wdt, tag="h8")
                nc.vector.tensor_mul(h8[:], base[:], m8t[:])

                # ---- G2 ------------------------------------------------
                g2x_eng = {"v": nc.vector, "g": nc.gpsimd}[plan.get("g2x",
                                                                    "v")]
                g2p_eng = {"v": nc.vector, "g": nc.gpsimd}[plan.get("g2p",
                                                                    "v")]
                x2 = small.tile([P, F, N], f32, tag="x2")
                g2x_eng.tensor_mul(
                    x2[:], eta_t[:],
                    dsq[:, 0:24].unsqueeze(1).broadcast_to([P, F, N]))
                e2b = small.tile([P, F, N], f32, tag="e2b")
                nc.scalar.activation(e2b[:], x2[:], Act.Exp, bias=ln05[:])
                if plan.get("g2_cat", False):
                    hgc = small.tile([P, 2, N], f32, tag="hgc")
                    nc.vector.tensor_mul(hgc[:, 0, :], bn[:, 0:24],
                                         m1t[:, 0:24])
                    nc.vector.tensor_mul(hgc[:, 1, :], bn[:, 0:24],
                                         m8t[:, 0:24])
                    prc = small.tile([P, 2, F, N], f32, tag="prc")
                    g2p_eng.tensor_mul(
                        prc[:],
                        e2b[:].unsqueeze(1).broadcast_to([P, 2, F, N]),
                        hgc[:].unsqueeze(2).broadcast_to([P, 2, F, N]))
                    nc.vector.reduce_sum(
                        out_t[:, 0:2 * F].rearrange("p (a f) -> p a f", a=2),
                        prc[:], axis=mybir.AxisListType.X)
                else:
                    hg1 = small.tile([P, N], f32, tag="hg1")
                    nc.vector.tensor_mul(hg1[:], bn[:, 0:24], m1t[:, 0:24])
                    hg8 = small.tile([P, N], f32, tag="hg8")
                    nc.vector.tensor_mul(hg8[:], bn[:, 0:24], m8t[:, 0:24])
                    pr1 = small.tile([P, F, N], f32, tag="pr1")
                    g2p_eng.tensor_mul(
                        pr1[:], e2b[:],
                        hg1[:].unsqueeze(1).broadcast_to([P, F, N]))
                    nc.vector.reduce_sum(out_t[:, 0:F], pr1[:],
                                         axis=mybir.AxisListType.X)
                    pr8 = small.tile([P, F, N], f32, tag="pr8")
                    g2p_eng.tensor_mul(
                        pr8[:], e2b[:],
                        hg8[:].unsqueeze(1).broadcast_to([P, F, N]))
                    nc.vector.reduce_sum(out_t[:, F:2 * F], pr8[:],
                                         axis=mybir.AxisListType.X)

                # ---- pair stage [P, 12, 24] ----------------------------
                shp = [P, ND, 24]
                ux, uy, uz = ud[:, 0, :], ud[:, 1, :], ud[:, 2, :]

                def jb2(sl):
                    return sl[:, 0:24].unsqueeze(1).broadcast_to(shp)

                def kb2(sl):
                    return bass.AP(sl.tensor, sl.offset + 1,
                                   [list(sl.ap[0]), [1, ND], [1, 24]])

                CXX = big.tile(shp, f32, tag="CXX")
                eng("cxx").tensor_mul(CXX[:], jb2(ux), kb2(ux))
                CYY = big.tile(shp, f32, tag="CYY")
                eng("cyy").tensor_mul(CYY[:], jb2(uy), kb2(uy))
                CZZ = big.tile(shp, f32, tag="CZZ")
                eng("czz").tensor_mul(CZZ[:], jb2(uz), kb2(uz))
                CCa = big.tile(shp, f32, tag="CCa")
                eng("cca").tensor_add(CCa[:], CXX[:], CYY[:])
                CC = big.tile(shp, f32, tag="CC")
                eng("cc").tensor_add(CC[:], CCa[:], CZZ[:])

                RP = big.tile(shp, f32, tag="RP")
                eng("rp").tensor_mul(RP[:], jb(dd), kb(dd))
                S = big.tile(shp, f32, tag="S")
                eng("s").tensor_add(S[:], jb(dsq), kb(dsq))
                T1 = big.tile(shp, f32, tag="T1")
                eng("t1").tensor_mul(T1[:], RP[:], CC[:])
                SQ = big.tile(shp, f32, tag="SQ")
                nc.vector.scalar_tensor_tensor(SQ[:], T1[:], -2.0, S[:],
                                               op0=Alu.mult, op1=Alu.add)
                TQ = big.tile(shp, f32, tag="TQ")
                eng("tq").tensor_scalar(TQ[:], SQ[:], 0.0, 1.0,
                                        Alu.max, Alu.min)
                S2Q = big.tile(shp, f32, tag="S2Q")
                nc.scalar.activation(S2Q[:], TQ[:], Act.Square)
                AQ = big.tile(shp, f32, tag="AQ")
                if plan["aq"] == "a":
                    nc.scalar.activation(AQ[:], TQ[:], Act.Copy,
                                         scale=float(c1), bias=float(c0))
                else:
                    nc.vector.tensor_scalar(AQ[:], TQ[:], c1, c0,
                                            Alu.mult, Alu.add)
                BQ = big.tile(shp, f32, tag="BQ")
                if plan["bq"] == "a":
                    nc.scalar.activation(BQ[:], TQ[:], Act.Copy,
                                         scale=float(c3), bias=float(c2))
                else:
                    nc.vector.tensor_scalar(BQ[:], TQ[:], c3, c2,
                                            Alu.mult, Alu.add)
                U1Q = big.tile(shp, f32, tag="U1Q")
                eng("u1q").tensor_mul(U1Q[:], S2Q[:], BQ[:])
                PSIQ = big.tile(shp, f32, tag="PSIQ")
                eng("psi").tensor_add(PSIQ[:], AQ[:], U1Q[:])
                G = big.tile(shp, wdt, tag="G")
                nc.scalar.activation(G[:], PSIQ[:], Act.Square,
                                     scale=float(np.sqrt(2.0)))

                W18a = big.tile(shp, wdt, tag="W18a")
                eng("w18a").tensor_mul(W18a[:], jb(h1), kb(h8))
                W18b = big.tile(shp, wdt, tag="W18b")
                eng("w18b").tensor_mul(W18b[:], jb(h8), kb(h1))
                W18 = big.tile(shp, wdt, tag="W18")
                eng("w18").tensor_add(W18[:], W18a[:], W18b[:])
                W11 = big.tile(shp, wdt, tag="W11")
                eng("w11").tensor_mul(W11[:], jb(h1), kb(h1))
                GW18 = big.tile(shp, wdt, tag="GW18")
                eng("gw18").tensor_mul(GW18[:], G[:], W18[:])
                GW11 = big.tile(shp, wdt, tag="GW11")
                eng("gw11").tensor_mul(GW11[:], G[:], W11[:])
                # d=12 diagonal double-counts its 12 unique pairs
                nc.vector.tensor_scalar_mul(GW18[:, ND - 1:ND, :],
                                            GW18[:, ND - 1:ND, :], 0.5)
                nc.vector.tensor_scalar_mul(GW11[:, ND - 1:ND, :],
                                            GW11[:, ND - 1:ND, :], 0.5)

                # ---- powers (1 +- c)^z via ACT Square chain ------------
                need = sorted({(1 if Lambda[f] > 0 else -1, zi[f])
                               for f in range(F)})
                pows = {}
                if plan.get("fuse_pows", False):
                    # paired [P,2,ND,24] tiles: slice 0 = (1+c)^z, 1 = (1-c)^z
                    maxz_all = max(zi)
                    if maxz_all >= 2:
                        pm = big.tile([P, 2, ND, 24], f32, tag="PM2")
                        nc.scalar.activation(pm[:, 0, :, :], CC[:],
                                             Act.Square, scale=1.0, bias=1.0)
                        nc.scalar.activation(pm[:, 1, :, :], CC[:],
                                             Act.Square, scale=-1.0, bias=1.0)
                        pows[(1, 2)] = pm[:, 0, :, :]
                        pows[(-1, 2)] = pm[:, 1, :, :]
                        z = 4
                        while z <= maxz_all:
                            nxt = big.tile([P, 2, ND, 24], f32, tag=f"PM{z}")
                            nc.scalar.activation(nxt[:], pm[:], Act.Square)
                            pows[(1, z)] = nxt[:, 0, :, :]
                            pows[(-1, z)] = nxt[:, 1, :, :]
                            pm = nxt
                            z *= 2
                    for sgn, z in need:
                        if z == 1 or (sgn, z) in pows:
                            continue
                        acc = None
                        bit = 2
                        rem = z & ~1
                        if z & 1:
                            raise ValueError("odd zeta>1 unsupported")
                        while rem:
                            if rem & bit:
                                term = pows[(sgn, bit)]
                                if acc is None:
                                    acc = term
                                else:
                                    na = big.tile([P, ND, 24], f32,
                                                  tag=f"pw{sgn}{z}{bit}")
                                    nc.vector.tensor_mul(na[:], acc[:],
                                                         term[:])
                                    acc = na[:]
                                rem &= ~bit
                            bit *= 2
                        pows[(sgn, z)] = acc
                sq_ct = [0]

                def mk_sq(dst, src, scale=1.0, bias=0.0):
                    ch = plan["sq_plan"][sq_ct[0] % len(plan["sq_plan"])]
                    sq_ct[0] += 1
                    if ch == "a":
                        nc.scalar.activation(dst[:], src[:], Act.Square,
                                             scale=float(scale),
                                             bias=float(bias))
                    else:
                        if bias != 0.0 or scale != 1.0:
                            tmp = big.tile(shp, f32, tag=f"sqt{sq_ct[0]}")
                            nc.vector.tensor_scalar(tmp[:], src[:],
                                                    float(scale),
                                                    float(bias),
                                                    Alu.mult, Alu.add)
                            src = tmp
                        nc.vector.tensor_mul(dst[:], src[:], src[:])

                maxz = {}
                if not plan.get("fuse_pows", False):
                    for sgn, z in need:
                        maxz[sgn] = max(maxz.get(sgn, 0), z)
                for sgn in maxz:
                    z = 2
                    prev = None
                    while z <= maxz[sgn]:
                        dst = big.tile(shp, f32, tag=f"pow{sgn}{z}")
                        if prev is None:
                            mk_sq(dst, CC, scale=float(sgn), bias=1.0)
                        else:
                            mk_sq(dst, prev)
                        pows[(sgn, z)] = dst
                        prev = dst
                        z *= 2
                # non-power-of-two z: build by binary decomposition
                for sgn, z in need:
                    if z == 1 or (sgn, z) in pows:
                        continue
                    acc = None
                    bit = 2
                    rem = z & ~1
                    if z & 1:
                        raise ValueError("odd zeta>1 unsupported in v2")
                    while rem:
                        if rem & bit:
                            term = pows[(sgn, bit)]
                            if acc is None:
                                acc = term
                            else:
                                na = big.tile(shp, f32, tag=f"pw{sgn}{z}{bit}")
                                nc.vector.tensor_mul(na[:], acc[:], term[:])
                                acc = na
                            rem &= ~bit
                        bit *= 2
                    pows[(sgn, z)] = acc

                # ---- accumulates ---------------------------------------
                scratch = big.tile(shp, f32, tag="scratch",
                                   bufs=plan.get("scratch_bufs", 2))
                for f in range(F):
                    sgn = 1 if Lambda[f] > 0 else -1
                    col18 = out_t[:, 2 * F + f:2 * F + f + 1]
                    col11 = out_t[:, 3 * F + f:3 * F + f + 1]
                    for GW, col, s_f in ((GW18, col18, sc[f]),
                                         (GW11, col11, sc[f])):
                        if zi[f] == 1 and plan["use_affine_z1"]:
                            nc.vector.affine_mul_reduce(
                                scratch[:], col, CC[:], GW[:],
                                float(sgn * s_f), float(s_f))
                        else:
                            Pf = pows[(sgn, zi[f])] if zi[f] > 1 else CC
                            if zi[f] == 1:
                                # (c*sgn*s + s) * GW via stt needs 2 ops;
                                # shouldn't happen with use_affine_z1
                                tmp = big.tile(shp, f32, tag="z1tmp")
                                nc.vector.tensor_scalar(
                                    tmp[:], CC[:], float(sgn * s_f),
                                    float(s_f), Alu.mult, Alu.add)
                                nc.vector.scalar_tensor_tensor(
                                    scratch[:], tmp[:], 1.0, GW[:],
                                    op0=Alu.mult, op1=Alu.mult,
                                    accum_out=col)
                            else:
                                pf_ap = Pf if isinstance(Pf, bass.AP) \
                                    else Pf[:]
                                if plan.get("acc_ttr", False):
                                    nc.vector.tensor_tensor_reduce(
                                        scratch[:], pf_ap, GW[:],
                                        float(s_f), 0.0, op0=Alu.mult,
                                        op1=Alu.add, accum_out=col)
                                else:
                                    nc.vector.scalar_tensor_tensor(
                                        scratch[:], pf_ap, float(s_f),
                                        GW[:], op0=Alu.mult, op1=Alu.mult,
                                        accum_out=col)

                nc.sync.dma_start(out_dr[r0:r1, :], out_t[:])

            if loop_reps:
                lk = {}
                if plan.get("loop_hints", False):
                    lk["hint_engines"] = (mybir.EngineType.DVE,
                                          mybir.EngineType.Activation)
                if plan.get("loop_staggered", False):
                    lk["staggered_reset"] = True
                loop_cm = tc.For_i(0, loop_reps, 1, **lk)
            else:
                loop_cm = contextlib.nullcontext()
            with loop_cm:
                for it in range(ntiles):
                    emit_tile(it)

    nc.compile()
    return nc


def _get_nc(key_arrays, loop_reps=None, plan=None):
    key = (tuple(np.asarray(a, np.float64).tobytes() for a in key_arrays)
           + (loop_reps, str(plan)))
    if key not in _BUILT:
        eta2, R_s, R_c2, zeta, Lambda, eta4, R_c4 = key_arrays
        _BUILT[key] = _build_nc(eta2, R_s, R_c2, zeta, Lambda,
                                float(eta4[0]), float(R_c4[0]),
                                loop_reps=loop_reps, plan=plan)
    return _BUILT[key]


def _host_prep(n_diff, n_dist, j_elems, rc):
    """Build per-core doubled/scaled input arrays."""
    wdtype = ml_dtypes.bfloat16 if PLAN["w_bf16"] else np.float32
    d = n_dist.reshape(A_TOT, N_NEI).astype(np.float32)
    u = n_diff.reshape(A_TOT, N_NEI, 3).astype(np.float32)
    u = u / d[..., None]
    dd = np.concatenate([d, d], axis=1) * np.float32(1.0 / rc)
    # ud layout per atom: [c, 48]
    ud = np.concatenate([u, u], axis=1).transpose(0, 2, 1)  # [A, 3, 48]
    ud = np.ascontiguousarray(ud).reshape(A_TOT, 6 * N_NEI)
    m1 = (j_elems == 1).astype(np.float32).reshape(A_TOT, N_NEI)
    m8 = (j_elems == 8).astype(np.float32).reshape(A_TOT, N_NEI)
    m1d = np.concatenate([m1, m1], axis=1).astype(wdtype)
    m8d = np.concatenate([m8, m8], axis=1).astype(wdtype)
    in_maps = []
    for c in range(N_CORES):
        s, e = c * A_CORE, (c + 1) * A_CORE
        in_maps.append({
            "dd": np.ascontiguousarray(dd[s:e]),
            "ud": np.ascontiguousarray(ud[s:e]),
            "m1": np.ascontiguousarray(m1d[s:e]),
            "m8": np.ascontiguousarray(m8d[s:e]),
        })
    return in_maps


def kernel(n_diff, n_dist, atom_i_idx, j_elems, eta2, R_s, R_c2,
           zeta, Lambda, eta4, R_c4, n_atoms, n_nei):
    n_diff = np.asarray(n_diff, np.float32)
    n_dist = np.asarray(n_dist, np.float32)
    atom_i_idx = np.asarray(atom_i_idx)
    j_elems = np.asarray(j_elems)
    eta2 = np.asarray(eta2, np.float32)
    R_s = np.asarray(R_s, np.float32)
    R_c2 = np.asarray(R_c2, np.float32)
    zeta = np.asarray(zeta, np.float32)
    Lambda = np.asarray(Lambda, np.float32)
    eta4 = np.asarray(eta4, np.float32)
    R_c4 = np.asarray(R_c4, np.float32)
    n_atoms = int(n_atoms)
    n_nei = int(n_nei)

    zi_ok = bool(np.allclose(zeta, np.round(zeta)) and np.all(zeta >= 1)
                 and all(int(z) == 1 or int(z) % 2 == 0 for z in zeta))
    idx_ok = bool(np.array_equal(
        atom_i_idx, np.repeat(np.arange(n_atoms, dtype=atom_i_idx.dtype),
                              n_nei)))
    shapes_ok = (n_atoms == A_TOT and n_nei == N_NEI and len(eta2) == F)
    uniform_ok = bool(np.all(eta4 == eta4[0]) and np.all(R_c4 == R_c4[0])
                      and np.all(R_c2 == R_c2[0])
                      and np.all(R_c2[0] == R_c4[0]) and np.all(R_s == 0.0)
                      and np.all(np.abs(Lambda) == 1.0))
    dist_ok = bool(np.all(n_dist > 1e-6) and np.all(n_dist < R_c4[0]))
    if not (zi_ok and idx_ok and shapes_ok and uniform_ok and dist_ok):
        return _np_reference(n_diff, n_dist, atom_i_idx, j_elems, eta2, R_s,
                             R_c2, zeta, Lambda, eta4, R_c4, n_atoms, n_nei)

    from concourse.bass_utils import run_bass_kernel_spmd

    nc = _get_nc((eta2, R_s, R_c2, zeta, Lambda, eta4, R_c4))
    in_maps = _host_prep(n_diff, n_dist, j_elems, float(R_c4[0]))
    res = run_bass_kernel_spmd(nc, in_maps, list(range(N_CORES)))
    return np.concatenate([res.results[c]["out"] for c in range(N_CORES)],
                          axis=0)
